# revision 6
# baseline (speedup 1.0000x reference)
"""AR(16) Gaussian log-likelihood kernel for Trainium2, 8 NeuronCores.

Math: out[b, t] = C - ((s[b,t] - sum_{k=1..16} phi_k s[b,t-k]) * invsc)^2
  with C = -0.5*log(2*pi*sigma^2), invsc = 1/(sqrt(2)*sigma).

Strategy (pure data parallel, 32 rows per core):
  - View each core's [32, 65536] shard as tiles of R=8 rows laid out on
    128 SBUF partitions with U = 4096 contiguous samples per partition
    (+32-sample halo so the causal window never crosses an AP boundary).
  - DVE stream-transpose (32x32 blocks) puts time-mod-32 on partitions.
  - TensorE computes q = (pred - s)*invsc as two banded-Toeplitz matmuls
    (within-block taps + previous-block corner taps) accumulated in PSUM,
    issued at 4 diagonal tile positions (K=32) so all four 32-partition
    groups run concurrently in the PE array.
  - ScalarE squares PSUM->SBUF, DVE applies C - x with a per-partition
    constant, DVE stream-transposes back, DMA out.
"""

import math

import numpy as np

import concourse.bass as bass
import concourse.tile as tile
from concourse import bacc, mybir
from concourse.bass_utils import run_bass_kernel_spmd

F32 = mybir.dt.float32
P = 16  # AR order
HALO = 32

# Full-problem constants (hardcoded; kernel.py must be self-contained)
B_FULL, T_FULL = 256, 65536
N_CORES = 8


def build_nc(b_core: int, t_len: int, rows_per_tile: int, win: int):
    """Build the single-core Bass program (SPMD: same program on all cores)."""
    R = rows_per_tile
    assert 128 % R == 0
    U = R * t_len // 128          # contiguous samples per partition
    cpr = 128 // R                # partitions per row; row starts at c % cpr == 0
    assert cpr * U == t_len
    ntiles = b_core // R
    assert ntiles * R == b_core
    W = min(win, U)
    assert U % W == 0
    nwin = U // W

    nc = bacc.Bacc(
        "TRN2", target_bir_lowering=False, debug=False, enable_asserts=False
    )
    s_h = nc.declare_dram_parameter("s", [b_core, t_len], F32, isOutput=False)
    tA_h = nc.declare_dram_parameter("toepA", [128, 32], F32, isOutput=False)
    tB_h = nc.declare_dram_parameter("toepB", [128, 32], F32, isOutput=False)
    cvec_h = nc.declare_dram_parameter("cvec", [128, 1], F32, isOutput=False)
    mask_h = nc.declare_dram_parameter("hmask", [128, 1], F32, isOutput=False)
    out_h = nc.declare_dram_parameter("out", [b_core, t_len], F32, isOutput=True)

    s_flat = s_h.ap().rearrange("b t -> (b t)")
    out_flat = out_h.ap().rearrange("b t -> (b t)")

    from contextlib import ExitStack

    with tile.TileContext(nc) as tc, ExitStack() as ctx:
        const_pool = ctx.enter_context(tc.tile_pool(name="const", bufs=1))
        in_pool = ctx.enter_context(tc.tile_pool(name="inp", bufs=2))
        st_pool = ctx.enter_context(tc.tile_pool(name="stp", bufs=2))
        sq_pool = ctx.enter_context(tc.tile_pool(name="sqp", bufs=2))
        out_pool = ctx.enter_context(tc.tile_pool(name="outp", bufs=2))
        psum_pool = ctx.enter_context(
            tc.tile_pool(name="psum", bufs=4, space="PSUM")
        )

        toepA = const_pool.tile([128, 32], F32)
        nc.sync.dma_start(out=toepA[:, :], in_=tA_h.ap())
        toepB = const_pool.tile([128, 32], F32)
        nc.sync.dma_start(out=toepB[:, :], in_=tB_h.ap())
        cvec = const_pool.tile([128, 1], F32)
        nc.sync.dma_start(out=cvec[:, :], in_=cvec_h.ap())
        hmask = const_pool.tile([128, 1], F32)
        nc.sync.dma_start(out=hmask[:, :], in_=mask_h.ap())

        for g in range(ntiles):
            base = g * 128 * U
            nat = in_pool.tile([128, U + HALO], F32, tag="nat")
            main_view = s_flat[base : base + 128 * U].rearrange(
                "(n u) -> n u", u=U
            )
            nc.sync.dma_start(out=nat[:, HALO:], in_=main_view)
            # halo: partition c gets flat[base + c*U - HALO : base + c*U)
            if g == 0:
                halo_view = s_flat[U - HALO : U - HALO + 127 * U].rearrange(
                    "(n u) -> n u", u=U
                )
                nc.sync.dma_start(out=nat[1:128, 0:HALO], in_=halo_view[:, 0:HALO])
                nc.vector.memset(nat[0:1, 0:HALO], 0.0)
            else:
                halo_view = s_flat[
                    base - HALO : base - HALO + 128 * U
                ].rearrange("(n u) -> n u", u=U)
                nc.sync.dma_start(out=nat[:, 0:HALO], in_=halo_view[:, 0:HALO])
            # zero the halo on row-start partitions (per-partition mask)
            nc.vector.tensor_scalar_mul(nat[:, 0:HALO], nat[:, 0:HALO], hmask[:, :])

            st = st_pool.tile([128, U + HALO], F32, tag="st")
            nc.vector.transpose(st[:, :], nat[:, :])

            sq = sq_pool.tile([128, U], F32, tag="sq")
            for w in range(nwin):
                f0 = w * W
                q = psum_pool.tile([128, W], F32, tag="q")
                for i in range(4):
                    pr = slice(32 * i, 32 * i + 32)
                    nc.tensor.matmul(
                        q[pr, :],
                        toepA[pr, :],
                        st[pr, f0 + 32 : f0 + 32 + W],
                        start=True,
                        stop=False,
                        tile_position=(32 * i, 32 * i),
                        skip_group_check=True,
                    )
                for i in range(4):
                    pr = slice(32 * i, 32 * i + 32)
                    nc.tensor.matmul(
                        q[pr, :],
                        toepB[pr, :],
                        st[pr, f0 : f0 + W],
                        start=False,
                        stop=True,
                        tile_position=(32 * i, 32 * i),
                        skip_group_check=True,
                    )
                nc.scalar.activation(
                    sq[:, f0 : f0 + W],
                    q[:, :],
                    mybir.ActivationFunctionType.Square,
                )

            aff = out_pool.tile([128, U], F32, tag="aff")
            nc.vector.tensor_scalar(
                aff[:, :],
                sq[:, :],
                -1.0,
                cvec[:, :],
                op0=mybir.AluOpType.mult,
                op1=mybir.AluOpType.add,
            )
            onat = out_pool.tile([128, U], F32, tag="onat")
            nc.vector.transpose(onat[:, :], aff[:, :])
            out_view = out_flat[base : base + 128 * U].rearrange(
                "(n u) -> n u", u=U
            )
            nc.sync.dma_start(out=out_view, in_=onat[:, :])

    nc.compile()
    return nc


def make_consts(coeffs: np.ndarray, noise_std: float):
    """Host-side O(1) prep: banded Toeplitz filter matrices + constants."""
    coeffs = np.asarray(coeffs, dtype=np.float64).reshape(-1)
    p = coeffs.shape[0]
    sigma = float(noise_std)
    invsc = 1.0 / (math.sqrt(2.0) * sigma)
    c_const = -0.5 * math.log(2.0 * math.pi * sigma * sigma)
    # taps h[0] = -invsc (the -s term), h[k] = invsc*phi_k
    h = np.zeros(p + 1, dtype=np.float64)
    h[0] = -invsc
    h[1:] = invsc * coeffs
    A = np.zeros((32, 32), dtype=np.float64)
    Bm = np.zeros((32, 32), dtype=np.float64)
    for pp in range(32):
        for m in range(32):
            d = m - pp
            if 0 <= d <= p:
                A[pp, m] = h[d]
            d2 = m - pp + 32
            if 1 <= d2 <= p:
                Bm[pp, m] = h[d2]
    toepA = np.tile(A, (4, 1)).astype(np.float32)
    toepB = np.tile(Bm, (4, 1)).astype(np.float32)
    cvec = np.full((128, 1), c_const, dtype=np.float32)
    return toepA, toepB, cvec


def make_hmask(rows_per_tile: int) -> np.ndarray:
    cpr = 128 // rows_per_tile
    m = np.ones((128, 1), dtype=np.float32)
    m[::cpr] = 0.0
    return m


_NC_CACHE: dict = {}


def _get_nc(b_core, t_len, rows_per_tile=8, win=512):
    key = (b_core, t_len, rows_per_tile, win)
    if key not in _NC_CACHE:
        _NC_CACHE[key] = build_nc(b_core, t_len, rows_per_tile, win)
    return _NC_CACHE[key]


def run_on_hw(s, coeffs, noise_std, rows_per_tile=8, win=512, trace=False,
              tmpdir=None):
    """Shard across 8 cores, run, gather. Returns (out, BassKernelResults)."""
    s = np.ascontiguousarray(np.asarray(s, dtype=np.float32))
    b_full, t_len = s.shape
    b_core = b_full // N_CORES
    nc = _get_nc(b_core, t_len, rows_per_tile, win)
    toepA, toepB, cvec = make_consts(coeffs, float(np.asarray(noise_std)))
    hmask = make_hmask(rows_per_tile)
    in_maps = [
        {
            "s": s[i * b_core : (i + 1) * b_core],
            "toepA": toepA,
            "toepB": toepB,
            "cvec": cvec,
            "hmask": hmask,
        }
        for i in range(N_CORES)
    ]
    res = run_bass_kernel_spmd(
        nc, in_maps, core_ids=list(range(N_CORES)), trace=trace, tmpdir=tmpdir
    )
    out = np.concatenate([res.results[i]["out"] for i in range(N_CORES)], axis=0)
    return out, res


def kernel(s, coeffs, noise_std):
    out, _ = run_on_hw(s, coeffs, noise_std)
    return out


# revision 9
# speedup vs baseline: 1.1709x; 1.1709x over previous
"""AR(16) Gaussian log-likelihood kernel for Trainium2, 8 NeuronCores.

Math: out[b, t] = C - ((s[b,t] - sum_{k=1..16} phi_k s[b,t-k]) * invsc)^2
  with C = -0.5*log(2*pi*sigma^2), invsc = 1/(sqrt(2)*sigma).

Strategy (pure data parallel, 32 rows per core):
  - View each core's [32, 65536] shard as tiles of R=8 rows laid out on
    128 SBUF partitions with U = 4096 contiguous samples per partition
    (+32-sample halo so the causal window never crosses an AP boundary).
  - DVE stream-transpose (32x32 blocks) puts time-mod-32 on partitions.
  - TensorE computes q = (pred - s)*invsc as two banded-Toeplitz matmuls
    (within-block taps + previous-block corner taps) accumulated in PSUM,
    issued at 4 diagonal tile positions (K=32) so all four 32-partition
    groups run concurrently in the PE array.
  - ScalarE squares PSUM->SBUF, DVE applies C - x with a per-partition
    constant, DVE stream-transposes back, DMA out.
"""

import math

import numpy as np

import concourse.bass as bass
import concourse.tile as tile
from concourse import bacc, mybir
from concourse.bass_utils import run_bass_kernel_spmd

F32 = mybir.dt.float32
BF16 = mybir.dt.bfloat16
P = 16  # AR order
HALO = 32

# Full-problem constants (hardcoded; kernel.py must be self-contained)
B_FULL, T_FULL = 256, 65536
N_CORES = 8


def build_nc(b_core: int, t_len: int, rows_per_tile: int, win: int):
    """Build the single-core Bass program (SPMD: same program on all cores)."""
    R = rows_per_tile
    assert 128 % R == 0
    U = R * t_len // 128          # contiguous samples per partition
    cpr = 128 // R                # partitions per row; row starts at c % cpr == 0
    assert cpr * U == t_len
    ntiles = b_core // R
    assert ntiles * R == b_core
    W = min(win, U)
    assert U % W == 0
    nwin = U // W

    nc = bacc.Bacc(
        "TRN2", target_bir_lowering=False, debug=False, enable_asserts=False
    )
    s_h = nc.declare_dram_parameter("s", [b_core, t_len], F32, isOutput=False)
    tA_h = nc.declare_dram_parameter("toepA", [128, 32], BF16, isOutput=False)
    tB_h = nc.declare_dram_parameter("toepB", [128, 32], BF16, isOutput=False)
    cvec_h = nc.declare_dram_parameter("cvec", [128, 1], F32, isOutput=False)
    mask_h = nc.declare_dram_parameter("hmask", [128, 1], F32, isOutput=False)
    out_h = nc.declare_dram_parameter("out", [b_core, t_len], F32, isOutput=True)

    s_flat = s_h.ap().rearrange("b t -> (b t)")
    out_flat = out_h.ap().rearrange("b t -> (b t)")

    from contextlib import ExitStack

    with tile.TileContext(nc) as tc, ExitStack() as ctx:
        const_pool = ctx.enter_context(tc.tile_pool(name="const", bufs=1))
        in_pool = ctx.enter_context(tc.tile_pool(name="inp", bufs=2))
        st_pool = ctx.enter_context(tc.tile_pool(name="stp", bufs=2))
        sq_pool = ctx.enter_context(tc.tile_pool(name="sqp", bufs=2))
        out_pool = ctx.enter_context(tc.tile_pool(name="outp", bufs=2))
        psum_pool = ctx.enter_context(
            tc.tile_pool(name="psum", bufs=4, space="PSUM")
        )

        toepA = const_pool.tile([128, 32], BF16)
        nc.sync.dma_start(out=toepA[:, :], in_=tA_h.ap())
        toepB = const_pool.tile([128, 32], BF16)
        nc.sync.dma_start(out=toepB[:, :], in_=tB_h.ap())
        cvec = const_pool.tile([128, 1], F32)
        nc.sync.dma_start(out=cvec[:, :], in_=cvec_h.ap())
        hmask = const_pool.tile([128, 1], F32)
        nc.sync.dma_start(out=hmask[:, :], in_=mask_h.ap())

        for g in range(ntiles):
            base = g * 128 * U
            nat = in_pool.tile([128, U + HALO], BF16, tag="nat")
            main_view = s_flat[base : base + 128 * U].rearrange(
                "(n u) -> n u", u=U
            )
            nc.gpsimd.dma_start(out=nat[:, HALO:], in_=main_view)
            # halo: partition c gets flat[base + c*U - HALO : base + c*U)
            if g == 0:
                halo_view = s_flat[U - HALO : U - HALO + 127 * U].rearrange(
                    "(n u) -> n u", u=U
                )
                nc.gpsimd.dma_start(out=nat[1:128, 0:HALO], in_=halo_view[:, 0:HALO])
                nc.vector.memset(nat[0:1, 0:HALO], 0.0)
            else:
                halo_view = s_flat[
                    base - HALO : base - HALO + 128 * U
                ].rearrange("(n u) -> n u", u=U)
                nc.gpsimd.dma_start(out=nat[:, 0:HALO], in_=halo_view[:, 0:HALO])
            # zero the halo on row-start partitions (per-partition mask)
            nc.vector.tensor_scalar_mul(nat[:, 0:HALO], nat[:, 0:HALO], hmask[:, :])

            st = st_pool.tile([128, U + HALO], BF16, tag="st")
            nc.vector.transpose(st[:, :], nat[:, :])

            sq = sq_pool.tile([128, U], F32, tag="sq")
            for w in range(nwin):
                f0 = w * W
                q = psum_pool.tile([128, W], F32, tag="q")
                for i in range(4):
                    pr = slice(32 * i, 32 * i + 32)
                    nc.tensor.matmul(
                        q[pr, :],
                        toepA[pr, :],
                        st[pr, f0 + 32 : f0 + 32 + W],
                        start=True,
                        stop=False,
                        tile_position=(32 * i, 32 * i),
                        skip_group_check=True,
                    )
                for i in range(4):
                    pr = slice(32 * i, 32 * i + 32)
                    nc.tensor.matmul(
                        q[pr, :],
                        toepB[pr, :],
                        st[pr, f0 : f0 + W],
                        start=False,
                        stop=True,
                        tile_position=(32 * i, 32 * i),
                        skip_group_check=True,
                    )
                nc.scalar.activation(
                    sq[:, f0 : f0 + W],
                    q[:, :],
                    mybir.ActivationFunctionType.Square,
                )

            aff = out_pool.tile([128, U], F32, tag="aff")
            nc.vector.tensor_scalar(
                aff[:, :],
                sq[:, :],
                -1.0,
                cvec[:, :],
                op0=mybir.AluOpType.mult,
                op1=mybir.AluOpType.add,
            )
            onat = out_pool.tile([128, U], F32, tag="onat")
            nc.vector.transpose(onat[:, :], aff[:, :])
            out_view = out_flat[base : base + 128 * U].rearrange(
                "(n u) -> n u", u=U
            )
            nc.sync.dma_start(out=out_view, in_=onat[:, :])

    nc.compile()
    return nc


def make_consts(coeffs: np.ndarray, noise_std: float):
    """Host-side O(1) prep: banded Toeplitz filter matrices + constants."""
    coeffs = np.asarray(coeffs, dtype=np.float64).reshape(-1)
    p = coeffs.shape[0]
    sigma = float(noise_std)
    invsc = 1.0 / (math.sqrt(2.0) * sigma)
    c_const = -0.5 * math.log(2.0 * math.pi * sigma * sigma)
    # taps h[0] = -invsc (the -s term), h[k] = invsc*phi_k
    h = np.zeros(p + 1, dtype=np.float64)
    h[0] = -invsc
    h[1:] = invsc * coeffs
    A = np.zeros((32, 32), dtype=np.float64)
    Bm = np.zeros((32, 32), dtype=np.float64)
    for pp in range(32):
        for m in range(32):
            d = m - pp
            if 0 <= d <= p:
                A[pp, m] = h[d]
            d2 = m - pp + 32
            if 1 <= d2 <= p:
                Bm[pp, m] = h[d2]
    import ml_dtypes
    toepA = np.tile(A, (4, 1)).astype(ml_dtypes.bfloat16)
    toepB = np.tile(Bm, (4, 1)).astype(ml_dtypes.bfloat16)
    cvec = np.full((128, 1), c_const, dtype=np.float32)
    return toepA, toepB, cvec


def make_hmask(rows_per_tile: int) -> np.ndarray:
    cpr = 128 // rows_per_tile
    m = np.ones((128, 1), dtype=np.float32)
    m[::cpr] = 0.0
    return m


_NC_CACHE: dict = {}


def _get_nc(b_core, t_len, rows_per_tile=8, win=512):
    key = (b_core, t_len, rows_per_tile, win)
    if key not in _NC_CACHE:
        _NC_CACHE[key] = build_nc(b_core, t_len, rows_per_tile, win)
    return _NC_CACHE[key]


def run_on_hw(s, coeffs, noise_std, rows_per_tile=8, win=512, trace=False,
              tmpdir=None):
    """Shard across 8 cores, run, gather. Returns (out, BassKernelResults)."""
    s = np.ascontiguousarray(np.asarray(s, dtype=np.float32))
    b_full, t_len = s.shape
    b_core = b_full // N_CORES
    nc = _get_nc(b_core, t_len, rows_per_tile, win)
    toepA, toepB, cvec = make_consts(coeffs, float(np.asarray(noise_std)))
    hmask = make_hmask(rows_per_tile)
    in_maps = [
        {
            "s": s[i * b_core : (i + 1) * b_core],
            "toepA": toepA,
            "toepB": toepB,
            "cvec": cvec,
            "hmask": hmask,
        }
        for i in range(N_CORES)
    ]
    res = run_bass_kernel_spmd(
        nc, in_maps, core_ids=list(range(N_CORES)), trace=trace, tmpdir=tmpdir
    )
    out = np.concatenate([res.results[i]["out"] for i in range(N_CORES)], axis=0)
    return out, res


def kernel(s, coeffs, noise_std):
    out, _ = run_on_hw(s, coeffs, noise_std)
    return out


# revision 10
# speedup vs baseline: 1.3810x; 1.1795x over previous
"""AR(16) Gaussian log-likelihood kernel for Trainium2, 8 NeuronCores.

Math: out[b, t] = C - ((s[b,t] - sum_{k=1..16} phi_k s[b,t-k]) * invsc)^2
  with C = -0.5*log(2*pi*sigma^2), invsc = 1/(sqrt(2)*sigma).

Strategy (pure data parallel, 32 rows per core):
  - View each core's [32, 65536] shard as tiles of R=8 rows laid out on
    128 SBUF partitions with U = 4096 contiguous samples per partition
    (+32-sample halo so the causal window never crosses an AP boundary).
  - DVE stream-transpose (32x32 blocks) puts time-mod-32 on partitions.
  - TensorE computes q = (pred - s)*invsc as two banded-Toeplitz matmuls
    (within-block taps + previous-block corner taps) accumulated in PSUM,
    issued at 4 diagonal tile positions (K=32) so all four 32-partition
    groups run concurrently in the PE array.
  - ScalarE squares PSUM->SBUF, DVE applies C - x with a per-partition
    constant, DVE stream-transposes back, DMA out.
"""

import math

import numpy as np

import concourse.bass as bass
import concourse.tile as tile
from concourse import bacc, mybir
from concourse.bass_utils import run_bass_kernel_spmd

F32 = mybir.dt.float32
BF16 = mybir.dt.bfloat16
P = 16  # AR order
HALO = 32

# Full-problem constants (hardcoded; kernel.py must be self-contained)
B_FULL, T_FULL = 256, 65536
N_CORES = 8


def build_nc(b_core: int, t_len: int, rows_per_tile: int, win: int):
    """Build the single-core Bass program (SPMD: same program on all cores)."""
    R = rows_per_tile
    assert 128 % R == 0
    U = R * t_len // 128          # contiguous samples per partition
    cpr = 128 // R                # partitions per row; row starts at c % cpr == 0
    assert cpr * U == t_len
    ntiles = b_core // R
    assert ntiles * R == b_core
    W = min(win, U)
    assert U % W == 0
    nwin = U // W

    nc = bacc.Bacc(
        "TRN2", target_bir_lowering=False, debug=False, enable_asserts=False
    )
    s_h = nc.declare_dram_parameter("s", [b_core, t_len], F32, isOutput=False)
    tA_h = nc.declare_dram_parameter("toepA", [128, 32], BF16, isOutput=False)
    tB_h = nc.declare_dram_parameter("toepB", [128, 32], BF16, isOutput=False)
    cvec_h = nc.declare_dram_parameter("cvec", [128, 1], F32, isOutput=False)
    mask_h = nc.declare_dram_parameter("hmask", [128, 1], F32, isOutput=False)
    out_h = nc.declare_dram_parameter("out", [b_core, t_len], F32, isOutput=True)

    s_flat = s_h.ap().rearrange("b t -> (b t)")
    out_flat = out_h.ap().rearrange("b t -> (b t)")

    from contextlib import ExitStack

    with tile.TileContext(nc) as tc, ExitStack() as ctx:
        const_pool = ctx.enter_context(tc.tile_pool(name="const", bufs=1))
        in_pool = ctx.enter_context(tc.tile_pool(name="inp", bufs=4))
        st_pool = ctx.enter_context(tc.tile_pool(name="stp", bufs=2))
        sq_pool = ctx.enter_context(tc.tile_pool(name="sqp", bufs=2))
        out_pool = ctx.enter_context(tc.tile_pool(name="outp", bufs=2))
        psum_pool = ctx.enter_context(
            tc.tile_pool(name="psum", bufs=4, space="PSUM")
        )

        toepA = const_pool.tile([128, 32], BF16)
        nc.sync.dma_start(out=toepA[:, :], in_=tA_h.ap())
        toepB = const_pool.tile([128, 32], BF16)
        nc.sync.dma_start(out=toepB[:, :], in_=tB_h.ap())
        cvec = const_pool.tile([128, 1], F32)
        nc.sync.dma_start(out=cvec[:, :], in_=cvec_h.ap())
        hmask = const_pool.tile([128, 1], F32)
        nc.sync.dma_start(out=hmask[:, :], in_=mask_h.ap())

        for g in range(ntiles):
            base = g * 128 * U
            nat = in_pool.tile([128, U + HALO], BF16, tag="nat")
            # halo: partition c gets flat[base + c*U - HALO : base + c*U)
            if g == 0:
                main_view = s_flat[base : base + 128 * U].rearrange(
                    "(n u) -> n u", u=U
                )
                nc.gpsimd.dma_start(out=nat[:, HALO:], in_=main_view)
                halo_view = bass.AP(s_h, U - HALO, [[U, 127], [1, HALO]])
                nc.gpsimd.dma_start(out=nat[1:128, 0:HALO], in_=halo_view)
                nc.vector.memset(nat[0:1, 0:HALO], 0.0)
            else:
                # one DMA: overlapping reads, partition stride U, row U+HALO
                ext_view = bass.AP(s_h, base - HALO, [[U, 128], [1, U + HALO]])
                nc.gpsimd.dma_start(out=nat[:, :], in_=ext_view)
            # zero the halo on row-start partitions (per-partition mask)
            nc.vector.tensor_scalar_mul(nat[:, 0:HALO], nat[:, 0:HALO], hmask[:, :])

            st = st_pool.tile([128, U + HALO], BF16, tag="st")
            nc.vector.transpose(st[:, :], nat[:, :])

            sq = sq_pool.tile([128, U], BF16, tag="sq")
            for w in range(nwin):
                f0 = w * W
                q = psum_pool.tile([128, W], F32, tag="q")
                for i in range(4):
                    pr = slice(32 * i, 32 * i + 32)
                    nc.tensor.matmul(
                        q[pr, :],
                        toepA[pr, :],
                        st[pr, f0 + 32 : f0 + 32 + W],
                        start=True,
                        stop=False,
                        tile_position=(32 * i, 32 * i),
                        skip_group_check=True,
                    )
                for i in range(4):
                    pr = slice(32 * i, 32 * i + 32)
                    nc.tensor.matmul(
                        q[pr, :],
                        toepB[pr, :],
                        st[pr, f0 : f0 + W],
                        start=False,
                        stop=True,
                        tile_position=(32 * i, 32 * i),
                        skip_group_check=True,
                    )
                nc.scalar.activation(
                    sq[:, f0 : f0 + W],
                    q[:, :],
                    mybir.ActivationFunctionType.Square,
                )

            aff = out_pool.tile([128, U], F32, tag="aff")
            nc.scalar.activation(
                aff[:, :],
                sq[:, :],
                mybir.ActivationFunctionType.Identity,
                bias=cvec[:, :],
                scale=-1.0,
            )
            onat = out_pool.tile([128, U], F32, tag="onat")
            nc.vector.transpose(onat[:, :], aff[:, :])
            out_view = out_flat[base : base + 128 * U].rearrange(
                "(n u) -> n u", u=U
            )
            nc.sync.dma_start(out=out_view, in_=onat[:, :])

    nc.compile()
    return nc


def make_consts(coeffs: np.ndarray, noise_std: float):
    """Host-side O(1) prep: banded Toeplitz filter matrices + constants."""
    coeffs = np.asarray(coeffs, dtype=np.float64).reshape(-1)
    p = coeffs.shape[0]
    sigma = float(noise_std)
    invsc = 1.0 / (math.sqrt(2.0) * sigma)
    c_const = -0.5 * math.log(2.0 * math.pi * sigma * sigma)
    # taps h[0] = -invsc (the -s term), h[k] = invsc*phi_k
    h = np.zeros(p + 1, dtype=np.float64)
    h[0] = -invsc
    h[1:] = invsc * coeffs
    A = np.zeros((32, 32), dtype=np.float64)
    Bm = np.zeros((32, 32), dtype=np.float64)
    for pp in range(32):
        for m in range(32):
            d = m - pp
            if 0 <= d <= p:
                A[pp, m] = h[d]
            d2 = m - pp + 32
            if 1 <= d2 <= p:
                Bm[pp, m] = h[d2]
    import ml_dtypes
    toepA = np.tile(A, (4, 1)).astype(ml_dtypes.bfloat16)
    toepB = np.tile(Bm, (4, 1)).astype(ml_dtypes.bfloat16)
    cvec = np.full((128, 1), c_const, dtype=np.float32)
    return toepA, toepB, cvec


def make_hmask(rows_per_tile: int) -> np.ndarray:
    cpr = 128 // rows_per_tile
    m = np.ones((128, 1), dtype=np.float32)
    m[::cpr] = 0.0
    return m


_NC_CACHE: dict = {}


def _get_nc(b_core, t_len, rows_per_tile=8, win=512):
    key = (b_core, t_len, rows_per_tile, win)
    if key not in _NC_CACHE:
        _NC_CACHE[key] = build_nc(b_core, t_len, rows_per_tile, win)
    return _NC_CACHE[key]


def run_on_hw(s, coeffs, noise_std, rows_per_tile=8, win=512, trace=False,
              tmpdir=None):
    """Shard across 8 cores, run, gather. Returns (out, BassKernelResults)."""
    s = np.ascontiguousarray(np.asarray(s, dtype=np.float32))
    b_full, t_len = s.shape
    b_core = b_full // N_CORES
    nc = _get_nc(b_core, t_len, rows_per_tile, win)
    toepA, toepB, cvec = make_consts(coeffs, float(np.asarray(noise_std)))
    hmask = make_hmask(rows_per_tile)
    in_maps = [
        {
            "s": s[i * b_core : (i + 1) * b_core],
            "toepA": toepA,
            "toepB": toepB,
            "cvec": cvec,
            "hmask": hmask,
        }
        for i in range(N_CORES)
    ]
    res = run_bass_kernel_spmd(
        nc, in_maps, core_ids=list(range(N_CORES)), trace=trace, tmpdir=tmpdir
    )
    out = np.concatenate([res.results[i]["out"] for i in range(N_CORES)], axis=0)
    return out, res


def kernel(s, coeffs, noise_std):
    out, _ = run_on_hw(s, coeffs, noise_std)
    return out


# revision 11
# speedup vs baseline: 1.3866x; 1.0040x over previous
"""AR(16) Gaussian log-likelihood kernel for Trainium2, 8 NeuronCores.

Math: out[b, t] = C - ((s[b,t] - sum_{k=1..16} phi_k s[b,t-k]) * invsc)^2
  with C = -0.5*log(2*pi*sigma^2), invsc = 1/(sqrt(2)*sigma).

Strategy (pure data parallel, 32 rows per core):
  - View each core's [32, 65536] shard as tiles of R=8 rows laid out on
    128 SBUF partitions with U = 4096 contiguous samples per partition
    (+32-sample halo so the causal window never crosses an AP boundary).
  - DVE stream-transpose (32x32 blocks) puts time-mod-32 on partitions.
  - TensorE computes q = (pred - s)*invsc as two banded-Toeplitz matmuls
    (within-block taps + previous-block corner taps) accumulated in PSUM,
    issued at 4 diagonal tile positions (K=32) so all four 32-partition
    groups run concurrently in the PE array.
  - ScalarE squares PSUM->SBUF, DVE applies C - x with a per-partition
    constant, DVE stream-transposes back, DMA out.
"""

import math

import numpy as np

import concourse.bass as bass
import concourse.tile as tile
from concourse import bacc, mybir
from concourse.bass_utils import run_bass_kernel_spmd

F32 = mybir.dt.float32
BF16 = mybir.dt.bfloat16
P = 16  # AR order
HALO = 32

# Full-problem constants (hardcoded; kernel.py must be self-contained)
B_FULL, T_FULL = 256, 65536
N_CORES = 8


def build_nc(b_core: int, t_len: int, rows_per_tile: int, win: int):
    """Build the single-core Bass program (SPMD: same program on all cores)."""
    R = rows_per_tile
    assert 128 % R == 0
    U = R * t_len // 128          # contiguous samples per partition
    cpr = 128 // R                # partitions per row; row starts at c % cpr == 0
    assert cpr * U == t_len
    ntiles = b_core // R
    assert ntiles * R == b_core
    W = min(win, U)
    assert U % W == 0
    nwin = U // W

    nc = bacc.Bacc(
        "TRN2", target_bir_lowering=False, debug=False, enable_asserts=False
    )
    s_h = nc.declare_dram_parameter("s", [b_core, t_len], F32, isOutput=False)
    tA_h = nc.declare_dram_parameter("toepA", [128, 32], BF16, isOutput=False)
    tB_h = nc.declare_dram_parameter("toepB", [128, 32], BF16, isOutput=False)
    cvec_h = nc.declare_dram_parameter("cvec", [128, 1], F32, isOutput=False)
    mask_h = nc.declare_dram_parameter("hmask", [128, 1], F32, isOutput=False)
    out_h = nc.declare_dram_parameter("out", [b_core, t_len], F32, isOutput=True)

    s_flat = s_h.ap().rearrange("b t -> (b t)")
    out_flat = out_h.ap().rearrange("b t -> (b t)")

    from contextlib import ExitStack

    with tile.TileContext(nc) as tc, ExitStack() as ctx:
        const_pool = ctx.enter_context(tc.tile_pool(name="const", bufs=1))
        in_pool = ctx.enter_context(tc.tile_pool(name="inp", bufs=4))
        st_pool = ctx.enter_context(tc.tile_pool(name="stp", bufs=2))
        sq_pool = ctx.enter_context(tc.tile_pool(name="sqp", bufs=2))
        out_pool = ctx.enter_context(tc.tile_pool(name="outp", bufs=2))
        psum_pool = ctx.enter_context(
            tc.tile_pool(name="psum", bufs=4, space="PSUM")
        )

        toepA = const_pool.tile([128, 32], BF16)
        nc.sync.dma_start(out=toepA[:, :], in_=tA_h.ap())
        toepB = const_pool.tile([128, 32], BF16)
        nc.sync.dma_start(out=toepB[:, :], in_=tB_h.ap())
        cvec = const_pool.tile([128, 1], F32)
        nc.sync.dma_start(out=cvec[:, :], in_=cvec_h.ap())
        hmask = const_pool.tile([128, 1], F32)
        nc.sync.dma_start(out=hmask[:, :], in_=mask_h.ap())

        for g in range(ntiles):
            base = g * 128 * U
            nat = in_pool.tile([128, U + HALO], BF16, tag="nat")
            # halo: partition c gets flat[base + c*U - HALO : base + c*U)
            if g == 0:
                main_view = s_flat[base : base + 128 * U].rearrange(
                    "(n u) -> n u", u=U
                )
                nc.gpsimd.dma_start(out=nat[:, HALO:], in_=main_view)
                halo_view = bass.AP(s_h, U - HALO, [[U, 127], [1, HALO]])
                nc.gpsimd.dma_start(out=nat[1:128, 0:HALO], in_=halo_view)
                nc.vector.memset(nat[0:1, 0:HALO], 0.0)
            else:
                # one DMA: overlapping reads, partition stride U, row U+HALO
                ext_view = bass.AP(s_h, base - HALO, [[U, 128], [1, U + HALO]])
                nc.gpsimd.dma_start(out=nat[:, :], in_=ext_view)
            # zero the halo on row-start partitions (per-partition mask)
            nc.vector.tensor_scalar_mul(nat[:, 0:HALO], nat[:, 0:HALO], hmask[:, :])

            st = st_pool.tile([128, U + HALO], BF16, tag="st")
            nc.vector.transpose(st[:, :], nat[:, :])

            sq = sq_pool.tile([128, U], BF16, tag="sq")
            for w in range(nwin):
                f0 = w * W
                q = psum_pool.tile([128, W], F32, tag="q")
                for i in range(4):
                    pr = slice(32 * i, 32 * i + 32)
                    nc.tensor.matmul(
                        q[pr, :],
                        toepA[pr, :],
                        st[pr, f0 + 32 : f0 + 32 + W],
                        start=True,
                        stop=False,
                        tile_position=(32 * i, 32 * i),
                        skip_group_check=True,
                    )
                for i in range(4):
                    pr = slice(32 * i, 32 * i + 32)
                    nc.tensor.matmul(
                        q[pr, :],
                        toepB[pr, :],
                        st[pr, f0 : f0 + W],
                        start=False,
                        stop=True,
                        tile_position=(32 * i, 32 * i),
                        skip_group_check=True,
                    )
                nc.scalar.activation(
                    sq[:, f0 : f0 + W],
                    q[:, :],
                    mybir.ActivationFunctionType.Square,
                )

            aff = out_pool.tile([128, U], F32, tag="aff")
            onat = out_pool.tile([128, U], F32, tag="onat")
            H = U // 2
            for h in range(2):
                hs = slice(h * H, (h + 1) * H)
                nc.gpsimd.tensor_scalar(
                    aff[:, hs],
                    sq[:, hs],
                    -1.0,
                    cvec[:, :],
                    op0=mybir.AluOpType.mult,
                    op1=mybir.AluOpType.add,
                )
                nc.vector.transpose(onat[:, hs], aff[:, hs])
                out_view = bass.AP(out_h, base + h * H, [[U, 128], [1, H]])
                nc.sync.dma_start(out=out_view, in_=onat[:, hs])

    nc.compile()
    return nc


def make_consts(coeffs: np.ndarray, noise_std: float):
    """Host-side O(1) prep: banded Toeplitz filter matrices + constants."""
    coeffs = np.asarray(coeffs, dtype=np.float64).reshape(-1)
    p = coeffs.shape[0]
    sigma = float(noise_std)
    invsc = 1.0 / (math.sqrt(2.0) * sigma)
    c_const = -0.5 * math.log(2.0 * math.pi * sigma * sigma)
    # taps h[0] = -invsc (the -s term), h[k] = invsc*phi_k
    h = np.zeros(p + 1, dtype=np.float64)
    h[0] = -invsc
    h[1:] = invsc * coeffs
    A = np.zeros((32, 32), dtype=np.float64)
    Bm = np.zeros((32, 32), dtype=np.float64)
    for pp in range(32):
        for m in range(32):
            d = m - pp
            if 0 <= d <= p:
                A[pp, m] = h[d]
            d2 = m - pp + 32
            if 1 <= d2 <= p:
                Bm[pp, m] = h[d2]
    import ml_dtypes
    toepA = np.tile(A, (4, 1)).astype(ml_dtypes.bfloat16)
    toepB = np.tile(Bm, (4, 1)).astype(ml_dtypes.bfloat16)
    cvec = np.full((128, 1), c_const, dtype=np.float32)
    return toepA, toepB, cvec


def make_hmask(rows_per_tile: int) -> np.ndarray:
    cpr = 128 // rows_per_tile
    m = np.ones((128, 1), dtype=np.float32)
    m[::cpr] = 0.0
    return m


_NC_CACHE: dict = {}


def _get_nc(b_core, t_len, rows_per_tile=8, win=512):
    key = (b_core, t_len, rows_per_tile, win)
    if key not in _NC_CACHE:
        _NC_CACHE[key] = build_nc(b_core, t_len, rows_per_tile, win)
    return _NC_CACHE[key]


def run_on_hw(s, coeffs, noise_std, rows_per_tile=8, win=512, trace=False,
              tmpdir=None):
    """Shard across 8 cores, run, gather. Returns (out, BassKernelResults)."""
    s = np.ascontiguousarray(np.asarray(s, dtype=np.float32))
    b_full, t_len = s.shape
    b_core = b_full // N_CORES
    nc = _get_nc(b_core, t_len, rows_per_tile, win)
    toepA, toepB, cvec = make_consts(coeffs, float(np.asarray(noise_std)))
    hmask = make_hmask(rows_per_tile)
    in_maps = [
        {
            "s": s[i * b_core : (i + 1) * b_core],
            "toepA": toepA,
            "toepB": toepB,
            "cvec": cvec,
            "hmask": hmask,
        }
        for i in range(N_CORES)
    ]
    res = run_bass_kernel_spmd(
        nc, in_maps, core_ids=list(range(N_CORES)), trace=trace, tmpdir=tmpdir
    )
    out = np.concatenate([res.results[i]["out"] for i in range(N_CORES)], axis=0)
    return out, res


def kernel(s, coeffs, noise_std):
    out, _ = run_on_hw(s, coeffs, noise_std)
    return out
